# revision 22
# baseline (speedup 1.0000x reference)
"""Trainium2 Bass kernel for a 2-layer GAT encoder (nn_Encoder_63273458205283).

Strategy (8 NeuronCores, full inputs in / full outputs out):
  - Host: append self-loops, degree-balance nodes into 128-node "windows"
    (49 windows per core, 392 total = 50176 slots), build a global node
    permutation.  Slots 0..25087 are the "lo" half, 25088..50175 "hi"
    (25088 = 196 windows exactly), so every window's dst slots live in one
    half and each half is int16-indexable for the SWDGE dma_gather ucode.
    Per window, edges are segregated by src half into K_LO "lo" tiles and
    K_HI "hi" tiles of 128 edges (padded with a dummy row of that half).
  - Device, per core:
      phase0: table1[slot] = [h0|h1|as|ad] = x @ [W1|v_src|v_dst] for ALL
              slots (bf16 rows of 384 elems = 768B, 256B-aligned for
              dma_gather); plus adtab1[own slot] = [as|ad] (256B rows).
      layer1: per window: batched dma_gather of src rows (lo+hi) and of
              per-edge dst attention rows from adtab1; ex =
              exp(leakyrelu(as_src + ad_dst)); per edge-tile: one-hot
              dst-scatter matrix S (shared across heads), messages scaled
              in place by ex with the ex pair appended (softmax denominator
              rides along as 2 extra matmul columns); one PSUM matmul per
              tile accumulates [msg0|msg1|sum_ex].  Normalize, bias, ELU;
              h1 @ [W2|v2] -> h2shard rows + adtab2 rows for own slots.
      AllGather h2shard -> h2table (512B rows).
      layer2: same edge pipeline on h2table/adtab2; ELU -> output rows.
  - Host: un-permute rows -> h2.  encoded output is x itself.
"""

import math
import os
from dataclasses import dataclass

import numpy as np

# ---------------- problem constants (hardcoded; kernel.py is self-contained)
N = 50000
E = 800000
IN = 128
H = 2
C1 = 128          # per-head dim of conv1
C2 = 64           # per-head dim of conv2
NEG_SLOPE = 0.2
NCORES = 8
NEG_BIG = -10000.0
HALF = 25088      # slots per table half (196 windows)
R1 = 384          # table1 row elems (768B bf16)
R2 = 256          # h2table row elems (512B bf16)
RA = 128          # adtab row elems (256B bf16)


@dataclass
class Cfg:
    n_cores: int = NCORES
    n_nodes: int = N
    in_dim: int = IN
    c1: int = C1
    c2: int = C2
    wpc: int = 49              # windows per core
    u_edge: int = 7            # edge-loop unroll (windows per For_i body)
    u0: int = 8                # phase0 unroll (node tiles per body)
    klo: int = 11              # lo edge tiles per window; set by prep
    khi: int = 10              # hi edge tiles per window; set by prep

    @property
    def k(self):
        return self.klo + self.khi

    @property
    def spc(self):             # slots per core
        return self.wpc * 128

    @property
    def n_slots(self):
        return self.n_cores * self.spc


# ---------------------------------------------------------------- host prep
def _pack_windows(deg: np.ndarray, n_windows: int, cap: np.ndarray):
    """LPT bin-packing of nodes into windows, balancing total degree."""
    import heapq

    order = np.argsort(-deg, kind="stable")
    heap = [(0, w) for w in range(n_windows)]
    heapq.heapify(heap)
    members: list[list[int]] = [[] for _ in range(n_windows)]
    for n in order:
        d = int(deg[n])
        load, w = heapq.heappop(heap)
        members[w].append(int(n))
        if len(members[w]) < cap[w]:
            heapq.heappush(heap, (load + d, w))
    return members


def _wrap_idxs(a: np.ndarray) -> np.ndarray:
    """[n] -> [128, n//16] int16: j -> (j%16, j//16), replicated x8."""
    w = a.reshape(-1, 16).T.astype(np.int16)   # [16, n//16]
    return np.ascontiguousarray(np.tile(w, (8, 1)))


def prep(cfg: Cfg, x, edge_index, W1, att_src1, att_dst1, b1, W2, att_src2,
         att_dst2, b2):
    import ml_dtypes
    bf = ml_dtypes.bfloat16
    nn = cfg.n_nodes
    n_slots = cfg.n_slots
    n_windows = cfg.n_cores * cfg.wpc
    spc = cfg.spc

    src = np.asarray(edge_index[0], dtype=np.int64)
    dst = np.asarray(edge_index[1], dtype=np.int64)
    loop = np.arange(nn, dtype=np.int64)
    src = np.concatenate([src, loop])
    dst = np.concatenate([dst, loop])

    deg = np.bincount(dst, minlength=nn)
    # reserve slot 25087 (win 195) and 50175 (win 391) as dummy rows
    cap = np.full(n_windows, 128, dtype=np.int64)
    cap[195] = 127
    cap[391] = 127
    members = _pack_windows(deg, n_windows, cap)

    pi = np.empty(nn, dtype=np.int64)
    for w, mem in enumerate(members):
        for j, node in enumerate(mem):
            pi[node] = w * 128 + j

    # pass 2: nodes keep their half; re-pack each half balancing the
    # (lo-src, hi-src) in-degree vector so K_LO/K_HI maxima shrink
    import heapq
    srch = pi[src] >= HALF
    dlo = np.bincount(dst[~srch], minlength=nn)
    dhi = np.bincount(dst[srch], minlength=nn)
    hw2 = n_windows // 2
    members = [[] for _ in range(n_windows)]
    for half in range(2):
        nodes = [n for n in range(nn)
                 if (pi[n] >= HALF) == bool(half)]
        nodes.sort(key=lambda n: -(dlo[n] + dhi[n]))
        base = half * hw2
        heap = [(0, 0, 0, base + w) for w in range(hw2)]
        heapq.heapify(heap)
        for n in nodes:
            _, lo, hi, w = heapq.heappop(heap)
            members[w].append(n)
            lo += int(dlo[n])
            hi += int(dhi[n])
            if len(members[w]) < cap[w]:
                heapq.heappush(heap, (max(lo, hi), lo, hi, w))
    # fix-up swaps: push any window side over the 9-tile boundary (1152)
    # under it by trading nodes with a same-half window that has slack
    LIM = 1152
    wlo = np.array([sum(dlo[n] for n in m) for m in members], dtype=np.int64)
    whi = np.array([sum(dhi[n] for n in m) for m in members], dtype=np.int64)
    # pad-slot self-loops land on the window's own half: tighten its limit
    padn = np.array([128 - len(m) for m in members], dtype=np.int64)
    padn[195] += 1
    padn[391] += 1
    wlo += np.where(np.arange(n_windows) < hw2, padn, 0)
    whi += np.where(np.arange(n_windows) >= hw2, padn, 0)
    for half in range(2):
        ws = range(half * hw2, (half + 1) * hw2)
        for _ in range(600):
            viol = [w for w in ws if wlo[w] > LIM or whi[w] > LIM]
            if not viol:
                break
            w = viol[0]
            dd, other = (dlo, dhi) if wlo[w] > LIM else (dhi, dlo)
            wn, wo = (wlo, whi) if wlo[w] > LIM else (whi, wlo)
            exc = wn[w] - LIM
            done = False
            for w2 in ws:
                if w2 == w or wn[w2] > LIM - exc:
                    continue
                for a in sorted(members[w], key=lambda n: -dd[n])[:8]:
                    for b in sorted(members[w2], key=lambda n: dd[n])[:8]:
                        d1 = dd[a] - dd[b]
                        d2 = other[a] - other[b]
                        if (d1 >= exc and wn[w2] + d1 <= LIM
                                and wo[w] + d2 <= LIM
                                and wo[w2] - d2 <= LIM):
                            ia = members[w].index(a)
                            ib = members[w2].index(b)
                            members[w][ia], members[w2][ib] = b, a
                            wn[w] -= d1; wn[w2] += d1
                            wo[w] += d2; wo[w2] -= d2
                            done = True
                            break
                    if done:
                        break
                if done:
                    break
            if not done:
                break
    for w, mem in enumerate(members):
        for j, node in enumerate(mem):
            pi[node] = w * 128 + j

    pad_slots = []
    for w, mem in enumerate(members):
        for j in range(len(mem), 128):
            pad_slots.append(w * 128 + j)
    pad_slots = np.asarray(pad_slots, dtype=np.int64)
    DLO, DHI = 25087, 50175          # dummy slots (one per half)
    # self loops for pad slots (incl. dummies) keep outputs finite
    esrc = np.concatenate([pi[src], pad_slots])
    edst = np.concatenate([pi[dst], pad_slots])

    ew = edst >> 7
    order = np.argsort(ew * 2 + (esrc >= HALF), kind="stable")
    esrc, edst, ew = esrc[order], edst[order], ew[order]
    is_hi = esrc >= HALF
    nlo = np.bincount(ew, weights=~is_hi, minlength=n_windows).astype(np.int64)
    nhi = np.bincount(ew, weights=is_hi, minlength=n_windows).astype(np.int64)
    cfg.klo = int(math.ceil(nlo.max() / 128))
    cfg.khi = int(math.ceil(nhi.max() / 128))
    KLO, KHI, K = cfg.klo, cfg.khi, cfg.k

    starts = np.zeros(n_windows + 1, dtype=np.int64)
    np.cumsum(nlo + nhi, out=starts[1:])
    j = np.arange(len(esrc)) - starts[ew]            # rank within window
    jlo = j                                          # rank within lo segment
    jhi = j - nlo[ew]                                # rank within hi segment
    # edge position within the window's K*128 slab: lo tiles then hi tiles
    pos = np.where(is_hi, KLO * 128 + jhi, jlo)
    flat = ew * (K * 128) + pos

    srcv = np.full(n_windows * K * 128, DLO, dtype=np.int64)
    # hi pad positions must point at the hi dummy
    slab = np.arange(n_windows * K * 128)
    khhh = (slab % (K * 128)) // 128
    srcv[khhh >= KLO] = DHI
    dstv = (slab // (K * 128)) * 128                 # window base slot
    dstv_arr = dstv.copy()
    locv = np.zeros(n_windows * K * 128, dtype=np.float64)
    srcv[flat] = esrc
    dstv_arr[flat] = edst
    locv[flat] = edst & 127

    srcv = srcv.reshape(n_windows, K, 128)
    dstv_arr = dstv_arr.reshape(n_windows, K, 128)
    locv = locv.reshape(n_windows, K, 128)

    # ---- permuted/transposed features
    x = np.asarray(x, dtype=np.float32)
    x_perm = np.zeros((n_slots, cfg.in_dim), dtype=np.float32)
    x_perm[pi] = x[:nn]
    xT = np.ascontiguousarray(x_perm.T).astype(bf)

    # ---- extended weights
    W1 = np.asarray(W1, np.float32)
    W2 = np.asarray(W2, np.float32)
    c1, c2 = cfg.c1, cfg.c2
    W1h = W1.reshape(cfg.in_dim, H, c1)
    vsrc1 = np.einsum("khc,hc->kh", W1h, np.asarray(att_src1, np.float32))
    vdst1 = np.einsum("khc,hc->kh", W1h, np.asarray(att_dst1, np.float32))
    wext1 = np.zeros((cfg.in_dim, 2 * c1 + 4), dtype=np.float32)
    wext1[:, 0:c1] = W1h[:, 0]
    wext1[:, c1:2 * c1] = W1h[:, 1]
    wext1[:, 2 * c1:2 * c1 + 2] = vsrc1
    wext1[:, 2 * c1 + 2:2 * c1 + 4] = vdst1
    wext1 = wext1.astype(bf)

    W2h = W2.reshape(2 * c1, H, c2)
    vsrc2 = np.einsum("khc,hc->kh", W2h, np.asarray(att_src2, np.float32))
    vdst2 = np.einsum("khc,hc->kh", W2h, np.asarray(att_dst2, np.float32))
    w2full = np.zeros((2 * c1, 2 * c2 + 4), dtype=np.float32)
    w2full[:, 0:c2] = W2h[:, 0]
    w2full[:, c2:2 * c2] = W2h[:, 1]
    w2full[:, 2 * c2:2 * c2 + 2] = vsrc2
    w2full[:, 2 * c2 + 2:2 * c2 + 4] = vdst2
    w2ext = np.ascontiguousarray(
        w2full.reshape(2, c1, 2 * c2 + 4)).astype(bf)

    b1r = np.tile(np.asarray(b1, np.float32)[None, :], (128, 1))
    b2r = np.tile(np.asarray(b2, np.float32)[None, :], (128, 1))
    iotaK = np.tile(np.arange(128, dtype=np.float32)[None, :],
                    (128, KLO + KHI)).astype(bf)
    ident = np.eye(128, dtype=np.float32)
    dummy1 = np.zeros((1, R1), dtype=np.float32)
    dummy1[0, 2 * c1:2 * c1 + 2] = NEG_BIG
    dummy1 = dummy1.astype(bf)
    dummy2 = np.zeros((1, R2), dtype=np.float32)
    dummy2[0, 2 * c2:2 * c2 + 2] = NEG_BIG
    dummy2 = dummy2.astype(bf)

    in_maps = []
    for c in range(cfg.n_cores):
        w0, w1_ = c * cfg.wpc, (c + 1) * cfg.wpc
        sv = srcv[w0:w1_]                       # [wpc, K, 128]
        dv = dstv_arr[w0:w1_] - c * spc         # own ordinals
        lv = locv[w0:w1_]
        # int16 gather index slabs, window-major rows
        lo = sv[:, :KLO, :].reshape(cfg.wpc, KLO * 128)
        hi = sv[:, KLO:, :].reshape(cfg.wpc, KHI * 128) - HALF
        ad = dv.reshape(cfg.wpc, K * 128)
        assert lo.min() >= 0 and lo.max() < 32768
        assert hi.min() >= 0 and hi.max() < 32768
        assert ad.min() >= 0 and ad.max() < spc
        idxlo = np.concatenate([_wrap_idxs(r) for r in lo], axis=0)
        idxhi = np.concatenate([_wrap_idxs(r) for r in hi], axis=0)
        dstloc = np.ascontiguousarray(
            lv.transpose(0, 2, 1).reshape(spc, K).astype(bf))
        eye = np.eye(128, dtype=bf)
        li = lv[:, :, :].astype(np.int64)       # [wpc, K, 128]
        stmat = np.ascontiguousarray(
            eye[:, li.reshape(cfg.wpc, K * 128)].transpose(1, 0, 2)
            .reshape(spc, K * 128) if False else
            np.concatenate([eye[:, li[w].reshape(-1)] for w in range(cfg.wpc)],
                           axis=0))
        in_maps.append({
            "xT": np.asarray(xT),
            "xTown": np.ascontiguousarray(xT[:, c * spc:(c + 1) * spc]),
            "wext1": np.asarray(wext1),
            "w2ext": np.asarray(w2ext),
            "b1r": b1r, "b2r": b2r,
            "iotaK": np.asarray(iotaK), "ident": ident,
            "dummy1": np.asarray(dummy1), "dummy2": np.asarray(dummy2),
            "idxlo": idxlo, "idxhi": idxhi,
            "dstloc": dstloc, "stmat": stmat,
        })
    return in_maps, pi


# ------------------------------------------------------------- bass builder
def build(cfg: Cfg):
    import concourse.bass as bass
    import concourse.bacc as bacc
    import concourse.mybir as mybir
    import concourse.tile as tile
    from concourse.bass import ds

    f32 = mybir.dt.float32
    bf16 = mybir.dt.bfloat16
    i16 = mybir.dt.int16
    Alu = mybir.AluOpType
    Act = mybir.ActivationFunctionType
    ET = mybir.EngineType

    KLO, KHI, K = cfg.klo, cfg.khi, cfg.k
    U, WPC = cfg.u_edge, cfg.wpc
    c1, c2 = cfg.c1, cfg.c2
    n_slots, spc = cfg.n_slots, cfg.spc
    NW = cfg.n_cores * WPC
    DLO, DHI = 25087, 50175

    nc = bacc.Bacc(num_devices=cfg.n_cores)

    # ---- I/O
    xT_d = nc.dram_tensor("xT", [cfg.in_dim, n_slots], bf16,
                          kind="ExternalInput")
    xTown_d = nc.dram_tensor("xTown", [cfg.in_dim, spc], bf16,
                             kind="ExternalInput")
    wext1_d = nc.dram_tensor("wext1", [cfg.in_dim, 2 * c1 + 4], bf16,
                             kind="ExternalInput")
    w2ext_d = nc.dram_tensor("w2ext", [2, c1, 2 * c2 + 4], bf16,
                             kind="ExternalInput")
    b1r_d = nc.dram_tensor("b1r", [128, 2 * c1], f32, kind="ExternalInput")
    b2r_d = nc.dram_tensor("b2r", [128, 2 * c2], f32, kind="ExternalInput")
    iotaK_d = nc.dram_tensor("iotaK", [128, (KLO + KHI) * 128], bf16,
                             kind="ExternalInput")
    ident_d = nc.dram_tensor("ident", [128, 128], f32, kind="ExternalInput")
    dummy1_d = nc.dram_tensor("dummy1", [1, R1], bf16, kind="ExternalInput")
    dummy2_d = nc.dram_tensor("dummy2", [1, R2], bf16, kind="ExternalInput")
    idxlo_d = nc.dram_tensor("idxlo", [spc, KLO * 8], i16,
                             kind="ExternalInput")
    idxhi_d = nc.dram_tensor("idxhi", [spc, KHI * 8], i16,
                             kind="ExternalInput")
    dstloc_d = nc.dram_tensor("dstloc", [spc, K], bf16, kind="ExternalInput")
    stmat_d = nc.dram_tensor("stmat", [spc, K * 128], bf16,
                             kind="ExternalInput")
    out2_d = nc.dram_tensor("out2", [spc, 2 * c2], f32, kind="ExternalOutput")

    table1 = nc.dram_tensor("table1", [n_slots, R1], bf16, kind="Internal")
    ownrows1 = nc.dram_tensor("ownrows1", [spc, 2 * c1 + 4], bf16,
                              kind="Internal")
    h2shard = nc.dram_tensor("h2shard", [spc, R2], bf16, kind="Internal")
    h2table = nc.dram_tensor("h2table", [n_slots, R2], bf16, kind="Internal")

    hint = (ET.DVE, ET.PE, ET.Activation)

    with tile.TileContext(nc) as tc:
        with (
            tc.tile_pool(name="const", bufs=1) as cpool,
            tc.tile_pool(name="work", bufs=3) as wpool,
            tc.tile_pool(name="small", bufs=6) as spool,
            tc.tile_pool(name="psum", bufs=2, space="PSUM") as ppool,
        ):
            # ---- load constants
            wext1_sb = cpool.tile([cfg.in_dim, 2 * c1 + 4], bf16, tag="wext1")
            nc.sync.dma_start(wext1_sb[:], wext1_d[:, :])
            w2ext_sb = cpool.tile([c1, 2, 2 * c2 + 4], bf16, tag="w2ext")
            nc.sync.dma_start(
                w2ext_sb[:], w2ext_d[:, :, :].rearrange("b p c -> p b c"))
            b1r_sb = cpool.tile([128, 2 * c1], f32, tag="b1r")
            nc.sync.dma_start(b1r_sb[:], b1r_d[:, :])
            b2r_sb = cpool.tile([128, 2 * c2], f32, tag="b2r")
            nc.sync.dma_start(b2r_sb[:], b2r_d[:, :])
            iotaK_sb = cpool.tile([128, K * 128], bf16, tag="iotaK")
            nc.sync.dma_start(iotaK_sb[:], iotaK_d[:, :])
            ident_sb = cpool.tile([128, 128], f32, tag="ident")
            nc.sync.dma_start(ident_sb[:], ident_d[:, :])
            dr1 = cpool.tile([1, R1], bf16, tag="dr1")
            nc.sync.dma_start(dr1[:], dummy1_d[:, :])
            dr2 = cpool.tile([1, R2], bf16, tag="dr2")
            nc.sync.dma_start(dr2[:], dummy2_d[:, :])

            # ---- phase 0: full layer1 table, replicated on every core
            u0 = cfg.u0
            assert (n_slots // 128) % u0 == 0
            with tc.For_i(0, n_slots, u0 * 128, hint_engines=hint) as i0:
                xsl = wpool.tile([cfg.in_dim, u0 * 128], bf16, tag="xsl")
                nc.sync.dma_start(xsl[:], xT_d[:, ds(i0, u0 * 128)])
                rsl = wpool.tile([128, u0, 2 * c1 + 4], bf16, tag="rsl")
                for u in range(u0):
                    ps0 = ppool.tile([128, 2 * c1 + 4], f32, tag="ps0",
                                     bufs=2)
                    nc.tensor.matmul(ps0[:], lhsT=xsl[:, u * 128:(u + 1) * 128],
                                     rhs=wext1_sb[:], start=True, stop=True)
                    nc.vector.tensor_copy(rsl[:, u, :], ps0[:])
                nc.sync.dma_start(
                    table1[ds(i0, u0 * 128), 0:2 * c1 + 4].rearrange(
                        "(u p) c -> p u c", p=128), rsl[:])

            # ---- own-slot rows for layer1 (feeds adw + keeps rows local)
            u1 = 7
            assert (spc // 128) % u1 == 0
            with tc.For_i(0, spc, u1 * 128, hint_engines=hint) as i1:
                xo = wpool.tile([cfg.in_dim, u1 * 128], bf16, tag="xo")
                nc.sync.dma_start(xo[:], xTown_d[:, ds(i1, u1 * 128)])
                ro = wpool.tile([128, u1, 2 * c1 + 4], bf16, tag="ro")
                for u in range(u1):
                    psa = ppool.tile([128, 2 * c1 + 4], f32, tag="ps0",
                                     bufs=2)
                    nc.tensor.matmul(
                        psa[:], lhsT=xo[:, u * 128:(u + 1) * 128],
                        rhs=wext1_sb[:], start=True, stop=True)
                    nc.vector.tensor_copy(ro[:, u, :], psa[:])
                nc.sync.dma_start(
                    ownrows1[ds(i1, u1 * 128), :].rearrange(
                        "(u p) c -> p u c", p=128), ro[:])

            # ---- dummy rows (after phase0: overwrite the reserved slots)
            nc.sync.dma_start(table1[DLO:DLO + 1, :], dr1[:])
            nc.sync.dma_start(table1[DHI:DHI + 1, :], dr1[:])

            # ---- shared edge phase
            def edge_phase(tlo, thi, own_d, ac, RG, C, bias_sb, finish):
                as_off = 2 * C
                with tc.For_i(0, spc, U * 128, hint_engines=hint) as iw:
                    locsl = wpool.tile([128, U, K], bf16, tag="locsl")
                    nc.sync.dma_start(
                        locsl[:],
                        dstloc_d[ds(iw, U * 128), :].rearrange(
                            "(u p) k -> p u k", p=128))
                    ilo = wpool.tile([128, U, KLO * 8], i16, tag="ilo")
                    nc.sync.dma_start(
                        ilo[:],
                        idxlo_d[ds(iw, U * 128), :].rearrange(
                            "(u p) k -> p u k", p=128))
                    ihi = wpool.tile([128, U, KHI * 8], i16, tag="ihi")
                    nc.sync.dma_start(
                        ihi[:],
                        idxhi_d[ds(iw, U * 128), :].rearrange(
                            "(u p) k -> p u k", p=128))
                    osl = wpool.tile([128, U, finish.out_w], finish.out_dt,
                                     tag="osl")
                    for u in range(U):
                        glo = wpool.tile([128, KLO, RG], bf16, tag="glo", bufs=5)
                        nc.gpsimd.dma_gather(
                            out_ap=glo[:], in_ap=tlo, idxs_ap=ilo[:, u, :],
                            num_idxs=KLO * 128, num_idxs_reg=KLO * 128,
                            elem_size=RG, single_packet=False)
                        ghi = wpool.tile([128, KHI, RG], bf16, tag="ghi", bufs=5)
                        nc.gpsimd.dma_gather(
                            out_ap=ghi[:], in_ap=thi, idxs_ap=ihi[:, u, :],
                            num_idxs=KHI * 128, num_idxs_reg=KHI * 128,
                            elem_size=RG, single_packet=False)
                        # transposed one-hots (host) + own-window att rows
                        stm = wpool.tile([128, K * 128], bf16, tag="stm", bufs=5)
                        nc.sync.dma_start(
                            stm[:], stmat_d[ds(iw + u * 128, 128), :])
                        adw = wpool.tile([128, 4], bf16, tag="adw")
                        nc.sync.dma_start(
                            adw[:], own_d[ds(iw + u * 128, 128), ac:ac + 4])
                        # per-edge dst attention via tiny matmuls
                        adps = ppool.tile([128, K, 2], f32, tag="adps",
                                          bufs=1)
                        for k in range(K):
                            nc.tensor.matmul(
                                adps[:, k, :],
                                lhsT=stm[:, k * 128:(k + 1) * 128],
                                rhs=adw[:, 2:4], start=True, stop=True)
                        # per-edge softmax numerators for all K tiles
                        e_t = spool.tile([128, K, 2], f32, tag="e")
                        nc.vector.tensor_tensor(
                            out=e_t[:, 0:KLO, :],
                            in0=glo[:, :, as_off:as_off + 2],
                            in1=adps[:, 0:KLO, :], op=Alu.add)
                        nc.vector.tensor_tensor(
                            out=e_t[:, KLO:K, :],
                            in0=ghi[:, :, as_off:as_off + 2],
                            in1=adps[:, KLO:K, :], op=Alu.add)
                        lr_t = spool.tile([128, K, 2], f32, tag="lr")
                        nc.vector.scalar_tensor_tensor(
                            out=lr_t[:], in0=e_t[:], scalar=NEG_SLOPE,
                            in1=e_t[:], op0=Alu.mult, op1=Alu.max)
                        ex_t = spool.tile([128, K, 2], bf16, tag="ex")
                        nc.scalar.activation(out=ex_t[:], in_=lr_t[:],
                                             func=Act.Exp)
                        # one-hot scatter matrices for all K tiles at once
                        s_all = spool.tile([128, K, 128], bf16, tag="S",
                                           bufs=2)
                        nc.vector.tensor_tensor(
                            out=s_all[:],
                            in0=iotaK_sb[:].rearrange(
                                "p (k j) -> p k j", k=K),
                            in1=locsl[:, u, :].unsqueeze(2).to_broadcast(
                                [128, K, 128]),
                            op=Alu.is_equal)
                        # scale all messages in place by ex (4D broadcast)
                        nc.vector.tensor_tensor(
                            out=glo[:, :, 0:2 * C].rearrange(
                                "p k (h j) -> p k h j", h=2),
                            in0=glo[:, :, 0:2 * C].rearrange(
                                "p k (h j) -> p k h j", h=2),
                            in1=ex_t[:, 0:KLO, :].unsqueeze(3).to_broadcast(
                                [128, KLO, 2, C]),
                            op=Alu.mult)
                        nc.vector.tensor_tensor(
                            out=ghi[:, :, 0:2 * C].rearrange(
                                "p k (h j) -> p k h j", h=2),
                            in0=ghi[:, :, 0:2 * C].rearrange(
                                "p k (h j) -> p k h j", h=2),
                            in1=ex_t[:, KLO:K, :].unsqueeze(3).to_broadcast(
                                [128, KHI, 2, C]),
                            op=Alu.mult)
                        acc = ppool.tile([128, 2 * C], f32, tag="acc",
                                         bufs=2)
                        acc2 = ppool.tile([128, 2], f32, tag="acc2", bufs=1)
                        for k in range(K):
                            gt, kk = (glo, k) if k < KLO else (ghi, k - KLO)
                            nc.tensor.matmul(
                                acc[:], lhsT=s_all[:, k, :],
                                rhs=gt[:, kk, 0:2 * C],
                                start=(k == 0), stop=(k == K - 1))
                            nc.tensor.matmul(
                                acc2[:],
                                lhsT=s_all[:, k, :], rhs=ex_t[:, k, :],
                                start=(k == 0), stop=(k == K - 1))
                        # window epilogue: normalize + bias + ELU
                        # (+1e-16 as in the reference: keeps empty segments
                        # finite so dummy slots never produce NaN)
                        sums = spool.tile([128, 2], f32, tag="sums")
                        nc.vector.tensor_scalar_add(
                            sums[:], acc2[:], 1e-16)
                        recip = spool.tile([128, 2], f32, tag="recip")
                        nc.vector.reciprocal(recip[:], sums[:])
                        ob = spool.tile([128, 2 * C], f32, tag="ob")
                        for h in range(2):
                            nc.vector.scalar_tensor_tensor(
                                out=ob[:, h * C:(h + 1) * C],
                                in0=acc[:, h * C:(h + 1) * C],
                                scalar=recip[:, h:h + 1],
                                in1=bias_sb[:, h * C:(h + 1) * C],
                                op0=Alu.mult, op1=Alu.add)
                        ee = spool.tile([128, 2 * C], f32, tag="ee")
                        nc.scalar.activation(out=ee[:], in_=ob[:],
                                             func=Act.Exp)
                        nc.vector.tensor_scalar_sub(ee[:], ee[:], 1.0)
                        mk = spool.tile([128, 2 * C], mybir.dt.uint8,
                                        tag="mk")
                        nc.vector.tensor_scalar(mk[:], ob[:], 0.0,
                                                scalar2=None, op0=Alu.is_gt)
                        nc.vector.copy_predicated(ee[:], mk[:], ob[:])
                        finish.emit(u, ee, osl)
                    finish.store(iw, osl)

            # ---- layer1 finish: build layer2 table rows for own slots
            class Fin1:
                out_w = 2 * c2 + 4
                out_dt = bf16

                def emit(self, u, ee, osl):
                    h1T = []
                    for b in range(2):
                        pst = ppool.tile([128, 128], f32, tag="pst", bufs=1)
                        nc.tensor.transpose(pst[:],
                                            ee[:, b * 128:(b + 1) * 128],
                                            ident_sb[:])
                        ht = wpool.tile([128, 128], bf16, tag=f"h1T{b}")
                        nc.vector.tensor_copy(ht[:], pst[:])
                        h1T.append(ht)
                    h2p = ppool.tile([128, 2 * c2 + 4], f32, tag="h2p",
                                     bufs=1)
                    nc.tensor.matmul(h2p[:], lhsT=h1T[0][:],
                                     rhs=w2ext_sb[:, 0, :], start=True,
                                     stop=False)
                    nc.tensor.matmul(h2p[:], lhsT=h1T[1][:],
                                     rhs=w2ext_sb[:, 1, :], start=False,
                                     stop=True)
                    nc.vector.tensor_copy(osl[:, u, :], h2p[:])

                def store(self, iw, osl):
                    nc.sync.dma_start(
                        h2shard[ds(iw, U * 128), 0:2 * c2 + 4].rearrange(
                            "(u p) c -> p u c", p=128), osl[:])

            # ---- layer2 finish: final output rows (f32)
            class Fin2:
                out_w = 2 * c2
                out_dt = f32

                def emit(self, u, ee, osl):
                    nc.vector.tensor_copy(osl[:, u, :], ee[:])

                def store(self, iw, osl):
                    nc.sync.dma_start(
                        out2_d[ds(iw, U * 128), :].rearrange(
                            "(u p) c -> p u c", p=128), osl[:])

            edge_phase(table1[0:HALF, 0:R1], table1[HALF:n_slots, 0:R1],
                       ownrows1, 2 * c1, R1, c1, b1r_sb, Fin1())

            nc.gpsimd.collective_compute(
                kind="AllGather", op=mybir.AluOpType.bypass,
                replica_groups=[list(range(cfg.n_cores))],
                ins=[h2shard[:, :]], outs=[h2table[0:n_slots, :]])

            # dummy rows for layer2 (after AllGather overwrote them)
            nc.sync.dma_start(h2table[DLO:DLO + 1, :], dr2[:])
            nc.sync.dma_start(h2table[DHI:DHI + 1, :], dr2[:])

            edge_phase(h2table[0:HALF, 0:R2], h2table[HALF:n_slots, 0:R2],
                       h2shard, 2 * c2, R2, c2, b2r_sb, Fin2())

    nc.finalize()
    return nc


# ------------------------------------------------------------------ driver
_CACHE: dict = {}


def kernel(x, edge_index, W1, att_src1, att_dst1, b1, W2, att_src2, att_dst2,
           b2):
    from concourse.bass_utils import run_bass_kernel_spmd

    cfg = Cfg()
    in_maps, pi = prep(cfg, x, edge_index, W1, att_src1, att_dst1, b1, W2,
                       att_src2, att_dst2, b2)
    key = (cfg.klo, cfg.khi)
    if key not in _CACHE:
        _CACHE[key] = build(cfg)
    nc = _CACHE[key]
    res = run_bass_kernel_spmd(nc, in_maps, core_ids=list(range(cfg.n_cores)))
    out = np.concatenate([res.results[c]["out2"] for c in range(cfg.n_cores)],
                         axis=0)
    h2 = np.ascontiguousarray(out[pi[:cfg.n_nodes]], dtype=np.float32)
    encoded = np.asarray(x, dtype=np.float32)
    return (h2, encoded)
